# revision 47
# baseline (speedup 1.0000x reference)
"""Trainium2 Bass kernel for nn_Conv2d_mvm (crossbar-quantized 3x3 conv).

The reference simulates a bit-sliced crossbar. Reductions:

1. The ADC clip [0, 511] can never bind (max per-xbar analog sum is
   128 rows * max slice digit 3 = 384), so the computation is exactly
   linear in the bit decompositions.

2. The weight reconstruction applies slice_w[0] = -2^14 to the whole
   MSB 2-bit digit, which is NOT true 2's complement: net effect the
   conv uses effective weights  w_eff = wi - 32768*[wi < 0]  with
   wi = rne(4096*w), and xi = rne(4096*x) exactly.

3. Precision slack (gate is rel_err < 2e-2; measured ~1.4e-3): skip
   integer rounding and fold the final /2^24 into the quant scales --
   fp16 is scale-invariant under powers of two, so
     xbuf = fp16(x)            (== fp16(4096 x) / 4096 exactly)
     weff = fp16(w - 8*[w < -1/8192])
   makes PSUM hold the final pre-clip value directly: no post scale
   pass; the DVE clips straight out of PSUM. fp16(x) is produced by
   the SWDGE cast-during-DMA, so the ACT engine runs no compute at
   all (no activation-table load on the critical path).

Implementation (8 cores, data-parallel over batch x row-blocks):
  - core c handles batch c//4, output rows 8*(c%4) .. 8*(c%4)+8.
  - K=128 tap packing: x lands twice -- partitions 0-63 the padded
    [10, 34] section flat, partitions 64-127 the same flat buffer
    shifted 35 columns (one row + one col). Taps with OFFS delta 35
    share one K=128 matmul (top half hits offset o, bottom o+35):
    pairs (0,4), (1,5), (3,7); taps 2, 6, 8 stay top-half K=64.
    6 matmul slots instead of 9, singles first (they only need the
    top x half + top-half weights, which land earliest).
    (A 5-slot variant with taps 6/7 as concurrent (0,0)/(64,0)
    row-tiles compiled but died at runtime -- base-partition-64
    K=64 matmuls are not usable.)
  - weights [128, 384] f32: col blocks 0-2 hold the pairs stacked
    top/bottom, blocks 3-5 taps 2, 6, 8 in the top half only.
  - DMA split: ACT carries w_top [64,384] (its doorbell is ~0.8us
    ahead of SP's, which sits behind a walrus-inserted drain), SP
    w_bot [64,192], GpSimd the two x cast-DMAs (f32 DRAM -> f16
    SBUF, top then bottom; SWDGE is the only caster). Separate
    semaphores per piece so consumers wait only for what they read.
  - DVE: mneg = -8*[w < -1/8192] (fp32 w -- the threshold must be
    evaluated pre-fp16 or ~5 mask bits flip), weff = fp16(w + mneg),
    singles chunk first (top w only), pairs chunk second. The x-cast
    waits live on the DVE behind each weff chunk (a tiny 8-col op
    carries the s_dve release after the x wait; then_inc directly on
    a wait_ge fires the set without honoring the wait). The PE only
    watches s_dve. After the matmuls the DVE clips
    v0 = fp16(min(max(psum, -8), 32767/4096)) straight out of PSUM
    in one full-width op (a single op pays the DVE PSUM-access
    overhead once); both out-DMA halves then issue together on
    SP/ACT (fini drains cover completion; the host upcast of the
    f16 output is exact, and saturated values -- ~95% of them --
    are f16-exact). GpSimd cannot read PSUM, so the clip cannot be
    split across engines.
  - PE warm-up: 7 garbage-input K=128 N=512 matmuls issue as the
    PE's first instructions (~3.1us at the cold 1.2GHz clock), so the
    HAM activity window un-throttles the PE clock to 2.4GHz right as
    the real matmuls (which start ~3.3us after main-block entry)
    begin. They read never-written SBUF into a scratch PSUM bank --
    no memsets or semaphores needed. (8 dummies overshoot and delay
    the real matmuls; fewer leave an idle hole that loses the ramp.)

All value arithmetic happens on device; the host only pads, shards,
reshapes and gathers.
"""

from contextlib import ExitStack

import numpy as np

import concourse.bass as bass
import concourse.mybir as mybir
from concourse.bass_utils import run_bass_kernel_spmd

# fixed problem shape
B, C, H, W = 2, 64, 32, 32
COUT = 64
RPC = 8                    # output rows per core
SECR = RPC + 2             # padded rows per section
SECW = W + 2               # padded width
LEN = SECR * SECW          # 340
NOUT = (RPC - 1) * SECW + W  # 270 psum columns covering all valid pixels
NW = 6 * COUT              # 384 packed weight cols (3 pairs + 3 singles)
NWB = 3 * COUT             # 192 bottom-half weight cols (pairs only)
NIN = LEN + NW             # 724 packed input columns per partition
NH = 136                   # out-DMA column split

AMAX = 32767.0 / 4096.0
AMIN = -8.0
THR = -1.0 / 8192.0        # wi = rne(4096 w) < 0  <=>  w < -1/8192

F32 = mybir.dt.float32
F16 = mybir.dt.float16

# The NEFF fini block resets every HW semaphore below the compiler's
# max-sem-num bound. Packing bass's kernel semaphores just above
# walrus's internal ones and telling walrus the bound keeps the
# program's semaphore footprint minimal.
MAX_SEM = 64


def _patch_sem_budget():
    import concourse.bass_utils as bu
    if getattr(bu, "_sem_budget_patched", False):
        return
    bass.get_walrus_max_sem_num = lambda: MAX_SEM - 8
    orig = bu.get_walrus_args

    def patched(*a, **k):
        # ldw-opt dedupes the identical LDWEIGHTS of the 7 warm-up
        # matmuls (smaller PE iram stream; real matmuls all have
        # distinct weights and are unaffected)
        return [*orig(*a, **k), f"--max-sem-num={MAX_SEM}",
                "--enable-ldw-opt=true"]

    bu.get_walrus_args = patched
    bu._sem_budget_patched = True


_CACHED = None


def _build():
    _patch_sem_budget()
    nc = bass.Bass("TRN2", target_bir_lowering=False, debug=False, num_devices=8,
                   monotonic_sem_count=0)
    main = nc.m.functions[0].blocks[0]
    assert main.name == "main"
    n_preamble = len(main.instructions)

    xwin = nc.dram_tensor("xw", [2 * C, NIN], F32, kind="ExternalInput").ap()
    yout = nc.dram_tensor("y", [COUT, NOUT], F16, kind="ExternalOutput").ap()

    with ExitStack() as ctx:
        wraw = ctx.enter_context(nc.sbuf_tensor([2 * C, NW], F32))
        xbuf = ctx.enter_context(nc.sbuf_tensor([2 * C, LEN], F16))
        mneg = ctx.enter_context(nc.sbuf_tensor([2 * C, NW], F16))
        weff = ctx.enter_context(nc.sbuf_tensor([2 * C, NW], F16))
        v0 = ctx.enter_context(nc.sbuf_tensor([COUT, NOUT], F16))
        wdum = ctx.enter_context(nc.sbuf_tensor([2 * C, COUT], F16))
        mdum = ctx.enter_context(nc.sbuf_tensor([2 * C, 512], F16))
        ps = ctx.enter_context(nc.psum_tensor([COUT, NOUT], F32))
        psd = ctx.enter_context(nc.psum_tensor([COUT, 512], F32))
        s_wt = ctx.enter_context(nc.semaphore())
        s_wb = ctx.enter_context(nc.semaphore())
        s_xt = ctx.enter_context(nc.semaphore())
        s_xb = ctx.enter_context(nc.semaphore())
        s_dve = ctx.enter_context(nc.semaphore())
        s_mm = ctx.enter_context(nc.semaphore())
        s_clip = ctx.enter_context(nc.semaphore())

        AL = mybir.AluOpType

        # ---- input DMAs: w halves on the two HWDGE queues (w_top on
        # the ACT queue -- the SP engine's first doorbell is ~0.7us
        # late behind a walrus-inserted drain), x via SWDGE cast-DMA
        # (f32 -> f16), top half first ----
        nc.scalar.dma_start(wraw[0:C, :], xwin[0:C, LEN:NIN]).then_inc(s_wt, 16)
        nc.sync.dma_start(wraw[C:2 * C, 0:NWB], xwin[C:2 * C, LEN:LEN + NWB]).then_inc(s_wb, 16)
        nc.gpsimd.dma_start(xbuf[0:C, :], xwin[0:C, 0:LEN]).then_inc(s_xt, 16)
        # bottom copy: pairs read cols 34..303 only
        nc.gpsimd.dma_start(xbuf[C:2 * C, 0:304], xwin[C:2 * C, 0:304]).then_inc(s_xb, 16)

        # ---- DVE weight chain: singles chunk (top-only) first. The
        # x-cast waits also live here so the PE only watches s_dve ----
        nc.vector.wait_ge(s_wt, 16)
        nc.vector.tensor_scalar(mneg[0:C, NWB:NW], wraw[0:C, NWB:NW], THR, -8.0, AL.is_lt, AL.mult)
        nc.vector.scalar_tensor_tensor(weff[0:C, NWB:NW], wraw[0:C, NWB:NW], 1.0, mneg[0:C, NWB:NW], AL.mult, AL.add)
        nc.vector.wait_ge(s_xt, 16)
        nc.vector.tensor_scalar(mneg[0:1, 0:8], wraw[0:1, 0:8], THR, -8.0, AL.is_lt, AL.mult).then_inc(s_dve, 1)
        nc.vector.wait_ge(s_wb, 16)
        nc.vector.tensor_scalar(mneg[:, 0:NWB], wraw[:, 0:NWB], THR, -8.0, AL.is_lt, AL.mult)
        nc.vector.scalar_tensor_tensor(weff[:, 0:NWB], wraw[:, 0:NWB], 1.0, mneg[:, 0:NWB], AL.mult, AL.add)
        nc.vector.wait_ge(s_xb, 16)
        nc.vector.tensor_scalar(mneg[0:1, 0:8], wraw[0:1, 0:8], THR, -8.0, AL.is_lt, AL.mult).then_inc(s_dve, 1)

        # ---- PE: warm-up dummies, then 6 real matmul slots ----
        NDUM = 7
        for i in range(NDUM):
            nc.tensor.matmul(psd[:], wdum[:, 0:COUT], mdum[:],
                             start=(i == 0), stop=(i == NDUM - 1))

        nc.tensor.wait_ge(s_dve, 1)
        for i, o in enumerate((2, 68, 70)):      # taps 2, 6, 8
            nc.tensor.matmul(ps[:], weff[0:C, (3 + i) * COUT:(4 + i) * COUT],
                             xbuf[0:C, o:o + NOUT], start=(i == 0), stop=False)
        nc.tensor.wait_ge(s_dve, 2)
        for i, o in enumerate((0, 1, 34)):       # pairs (0,4), (1,5), (3,7)
            mm = nc.tensor.matmul(ps[:], weff[:, i * COUT:(i + 1) * COUT],
                                  xbuf[:, o:o + NOUT], start=False, stop=(i == 2))
        mm.then_inc(s_mm, 1)

        # ---- post: one full-width clip straight out of PSUM (a
        # single op pays the DVE PSUM-access overhead once); both
        # out-DMA halves issue together on SP/ACT (fini drains cover
        # completion) ----
        nc.vector.wait_ge(s_mm, 1)
        nc.vector.tensor_scalar(v0[:], ps[:], AMAX, AMIN, AL.min, AL.max).then_inc(s_clip, 1)
        # ONE full-width out-DMA: issue cost is per-descriptor
        # (= partitions), so splitting buys nothing once both halves
        # wait on the same clip -- it only doubled the queue-drain
        # exposure on the barrier gate
        nc.sync.wait_ge(s_clip, 1)
        nc.sync.dma_start(yout[:], v0[:]).then_inc(s_wt, 16)

    # Strip the framework const-AP memsets and the post-init all-engine
    # barrier (they are unused here; HW semaphores are zero at NEFF load
    # and re-zeroed by the NEFF epilogue). Only the construction-time
    # preamble prefix is touched.
    insts = main.instructions
    pre = [
        ins for ins in insts[:n_preamble]
        if type(ins).__name__ not in (
            "InstMemset", "InstDrain", "InstEventSemaphore", "InstRegisterMove")
    ]
    main.instructions = pre + insts[n_preamble:]

    return nc


def _get_nc():
    global _CACHED
    if _CACHED is None:
        _CACHED = _build()
    return _CACHED


def _shard_inputs(x, weight):
    xpad = np.pad(np.ascontiguousarray(x, dtype=np.float32),
                  ((0, 0), (0, 0), (1, 1), (1, 1)))
    wre = np.asarray(weight, dtype=np.float32).transpose(1, 2, 3, 0)  # [ci, kh, kw, co]
    wtap = [wre[:, t // 3, t % 3, :] for t in range(9)]               # each [C, COUT]
    w_top = np.concatenate([wtap[0], wtap[1], wtap[3], wtap[2], wtap[6], wtap[8]], axis=1)
    w_bot = np.concatenate([wtap[4], wtap[5], wtap[7]], axis=1)
    in_maps = []
    for c in range(8):
        b, q = divmod(c, 4)
        top = xpad[b, :, RPC * q:RPC * q + SECR, :].reshape(C, LEN)
        xw = np.zeros((2 * C, NIN), np.float32)
        xw[0:C, 0:LEN] = top
        xw[C:2 * C, 0:LEN - 35] = top[:, 35:]
        xw[0:C, LEN:NIN] = w_top
        xw[C:2 * C, LEN:LEN + NWB] = w_bot
        in_maps.append({"xw": np.ascontiguousarray(xw)})
    return in_maps


def kernel(x, weight):
    nc = _get_nc()
    in_maps = _shard_inputs(x, weight)
    res = run_bass_kernel_spmd(nc, in_maps, core_ids=list(range(8)))
    out = np.empty((B, COUT, H, W), dtype=np.float32)
    for c in range(8):
        b, q = divmod(c, 4)
        y = np.asarray(res.results[c]["y"], dtype=np.float32)  # exact f16 upcast
        for r in range(RPC):
            out[b, :, RPC * q + r, :] = y[:, r * SECW:r * SECW + W]
    return out


# revision 48
# speedup vs baseline: 1.1042x; 1.1042x over previous
"""Trainium2 Bass kernel for nn_Conv2d_mvm (crossbar-quantized 3x3 conv).

The reference simulates a bit-sliced crossbar. Reductions:

1. The ADC clip [0, 511] can never bind (max per-xbar analog sum is
   128 rows * max slice digit 3 = 384), so the computation is exactly
   linear in the bit decompositions.

2. The weight reconstruction applies slice_w[0] = -2^14 to the whole
   MSB 2-bit digit, which is NOT true 2's complement: net effect the
   conv uses effective weights  w_eff = wi - 32768*[wi < 0]  with
   wi = rne(4096*w), and xi = rne(4096*x) exactly.

3. Precision slack (gate is rel_err < 2e-2; measured ~1.4e-3): skip
   integer rounding and fold the final /2^24 into the quant scales --
   fp16 is scale-invariant under powers of two, so
     xbuf = fp16(x)            (== fp16(4096 x) / 4096 exactly)
     weff = fp16(w - 8*[w < -1/8192])
   makes PSUM hold the final pre-clip value directly: no post scale
   pass; the DVE clips straight out of PSUM. fp16(x) is produced by
   the SWDGE cast-during-DMA, so the ACT engine runs no compute at
   all (no activation-table load on the critical path).

Implementation (8 cores, data-parallel over batch x row-blocks):
  - core c handles batch c//4, output rows 8*(c%4) .. 8*(c%4)+8.
  - K=128 tap packing: x lands twice -- partitions 0-63 the padded
    [10, 34] section flat, partitions 64-127 the same flat buffer
    shifted 35 columns (one row + one col). Taps with OFFS delta 35
    share one K=128 matmul (top half hits offset o, bottom o+35):
    pairs (0,4), (1,5), (3,7); taps 2, 6, 8 stay top-half K=64.
    6 matmul slots instead of 9, singles first (they only need the
    top x half + top-half weights, which land earliest).
    (A 5-slot variant with taps 6/7 as concurrent (0,0)/(64,0)
    row-tiles compiled but died at runtime -- base-partition-64
    K=64 matmuls are not usable.)
  - weights [128, 384] f32: col blocks 0-2 hold the pairs stacked
    top/bottom, blocks 3-5 taps 2, 6, 8 in the top half only.
  - DMA split: ACT carries w_top [64,384] (its doorbell is ~0.8us
    ahead of SP's, which sits behind a walrus-inserted drain), SP
    w_bot [64,192], GpSimd the two x cast-DMAs (f32 DRAM -> f16
    SBUF, top then bottom; SWDGE is the only caster). Separate
    semaphores per piece so consumers wait only for what they read.
  - DVE: mneg = -8*[w < -1/8192] (fp32 w -- the threshold must be
    evaluated pre-fp16 or ~5 mask bits flip), weff = fp16(w + mneg),
    singles chunk first (top w only), pairs chunk second. The x-cast
    waits live on the DVE behind each weff chunk (a tiny 8-col op
    carries the s_dve release after the x wait; then_inc directly on
    a wait_ge fires the set without honoring the wait). The PE only
    watches s_dve. After the matmuls the DVE clips
    v0 = fp16(min(max(psum, -8), 32767/4096)) straight out of PSUM
    in one full-width op (a single op pays the DVE PSUM-access
    overhead once); both out-DMA halves then issue together on
    SP/ACT (fini drains cover completion; the host upcast of the
    f16 output is exact, and saturated values -- ~95% of them --
    are f16-exact). GpSimd cannot read PSUM, so the clip cannot be
    split across engines.
  - PE warm-up: 7 garbage-input K=128 N=512 matmuls issue as the
    PE's first instructions (~3.1us at the cold 1.2GHz clock), so the
    HAM activity window un-throttles the PE clock to 2.4GHz right as
    the real matmuls (which start ~3.3us after main-block entry)
    begin. They read never-written SBUF into a scratch PSUM bank --
    no memsets or semaphores needed. (8 dummies overshoot and delay
    the real matmuls; fewer leave an idle hole that loses the ramp.)

All value arithmetic happens on device; the host only pads, shards,
reshapes and gathers.
"""

from contextlib import ExitStack

import numpy as np

import concourse.bass as bass
import concourse.mybir as mybir
from concourse.bass_utils import run_bass_kernel_spmd

# fixed problem shape
B, C, H, W = 2, 64, 32, 32
COUT = 64
RPC = 8                    # output rows per core
SECR = RPC + 2             # padded rows per section
SECW = W + 2               # padded width
LEN = SECR * SECW          # 340
NOUT = (RPC - 1) * SECW + W  # 270 psum columns covering all valid pixels
NW = 6 * COUT              # 384 packed weight cols (3 pairs + 3 singles)
NWB = 3 * COUT             # 192 bottom-half weight cols (pairs only)
NIN = LEN + NW             # 724 packed input columns per partition
NH = 136                   # out-DMA column split

AMAX = 32767.0 / 4096.0
AMIN = -8.0
THR = -1.0 / 8192.0        # wi = rne(4096 w) < 0  <=>  w < -1/8192

F32 = mybir.dt.float32
F16 = mybir.dt.float16

# The NEFF fini block resets every HW semaphore below the compiler's
# max-sem-num bound. Packing bass's kernel semaphores just above
# walrus's internal ones and telling walrus the bound keeps the
# program's semaphore footprint minimal.
MAX_SEM = 64


def _patch_sem_budget():
    import concourse.bass_utils as bu
    if getattr(bu, "_sem_budget_patched", False):
        return
    bass.get_walrus_max_sem_num = lambda: MAX_SEM - 8
    orig = bu.get_walrus_args

    def patched(*a, **k):
        # ldw-opt dedupes the identical LDWEIGHTS of the 7 warm-up
        # matmuls (smaller PE iram stream; real matmuls all have
        # distinct weights and are unaffected)
        return [*orig(*a, **k), f"--max-sem-num={MAX_SEM}",
                "--enable-ldw-opt=true"]

    bu.get_walrus_args = patched
    bu._sem_budget_patched = True


_CACHED = None


def _build():
    _patch_sem_budget()
    nc = bass.Bass("TRN2", target_bir_lowering=False, debug=False, num_devices=8,
                   monotonic_sem_count=0)
    main = nc.m.functions[0].blocks[0]
    assert main.name == "main"
    n_preamble = len(main.instructions)

    xwin = nc.dram_tensor("xw", [2 * C, NIN], F32, kind="ExternalInput").ap()
    yout = nc.dram_tensor("y", [COUT, NOUT], F16, kind="ExternalOutput").ap()

    with ExitStack() as ctx:
        wraw = ctx.enter_context(nc.sbuf_tensor([2 * C, NW], F32))
        xbuf = ctx.enter_context(nc.sbuf_tensor([2 * C, LEN], F16))
        mneg = ctx.enter_context(nc.sbuf_tensor([2 * C, NW], F16))
        weff = ctx.enter_context(nc.sbuf_tensor([2 * C, NW], F16))
        v0 = ctx.enter_context(nc.sbuf_tensor([COUT, NOUT], F16))
        wdum = ctx.enter_context(nc.sbuf_tensor([2 * C, COUT], F16))
        mdum = ctx.enter_context(nc.sbuf_tensor([2 * C, 512], F16))
        ps = ctx.enter_context(nc.psum_tensor([COUT, NOUT], F32))
        psd = ctx.enter_context(nc.psum_tensor([COUT, 512], F32))
        s_wt = ctx.enter_context(nc.semaphore())
        s_wb = ctx.enter_context(nc.semaphore())
        s_xt = ctx.enter_context(nc.semaphore())
        s_xb = ctx.enter_context(nc.semaphore())
        s_dve = ctx.enter_context(nc.semaphore())
        s_mm = ctx.enter_context(nc.semaphore())
        s_clip = ctx.enter_context(nc.semaphore())

        AL = mybir.AluOpType

        # ---- input DMAs: w halves on the two HWDGE queues (w_top on
        # the ACT queue -- the SP engine's first doorbell is ~0.7us
        # late behind a walrus-inserted drain), x via SWDGE cast-DMA
        # (f32 -> f16), top half first ----
        nc.scalar.dma_start(wraw[0:C, :], xwin[0:C, LEN:NIN]).then_inc(s_wt, 16)
        nc.sync.dma_start(wraw[C:2 * C, 0:NWB], xwin[C:2 * C, LEN:LEN + NWB]).then_inc(s_wb, 16)
        nc.gpsimd.dma_start(xbuf[0:C, :], xwin[0:C, 0:LEN]).then_inc(s_xt, 16)
        # bottom copy: pairs read cols 34..303 only
        nc.gpsimd.dma_start(xbuf[C:2 * C, 0:304], xwin[C:2 * C, 0:304]).then_inc(s_xb, 16)

        # ---- DVE weight chain: singles chunk (top-only) first. The
        # x-cast waits also live here so the PE only watches s_dve ----
        nc.vector.wait_ge(s_wt, 16)
        nc.vector.tensor_scalar(mneg[0:C, NWB:NW], wraw[0:C, NWB:NW], THR, -8.0, AL.is_lt, AL.mult)
        nc.vector.scalar_tensor_tensor(weff[0:C, NWB:NW], wraw[0:C, NWB:NW], 1.0, mneg[0:C, NWB:NW], AL.mult, AL.add)
        nc.vector.wait_ge(s_xt, 16)
        nc.vector.tensor_scalar(mneg[0:1, 0:8], wraw[0:1, 0:8], THR, -8.0, AL.is_lt, AL.mult).then_inc(s_dve, 1)
        nc.vector.wait_ge(s_wb, 16)
        nc.vector.tensor_scalar(mneg[:, 0:NWB], wraw[:, 0:NWB], THR, -8.0, AL.is_lt, AL.mult)
        nc.vector.scalar_tensor_tensor(weff[:, 0:NWB], wraw[:, 0:NWB], 1.0, mneg[:, 0:NWB], AL.mult, AL.add)
        nc.vector.wait_ge(s_xb, 16)
        nc.vector.tensor_scalar(mneg[0:1, 0:8], wraw[0:1, 0:8], THR, -8.0, AL.is_lt, AL.mult).then_inc(s_dve, 1)

        # ---- PE: warm-up dummies, then 6 real matmul slots ----
        NDUM = 7
        for i in range(NDUM):
            nc.tensor.matmul(psd[:], wdum[:, 0:COUT], mdum[:],
                             start=(i == 0), stop=(i == NDUM - 1))

        nc.tensor.wait_ge(s_dve, 1)
        for i, o in enumerate((2, 68, 70)):      # taps 2, 6, 8
            nc.tensor.matmul(ps[:], weff[0:C, (3 + i) * COUT:(4 + i) * COUT],
                             xbuf[0:C, o:o + NOUT], start=(i == 0), stop=False)
        nc.tensor.wait_ge(s_dve, 2)
        for i, o in enumerate((0, 1, 34)):       # pairs (0,4), (1,5), (3,7)
            mm = nc.tensor.matmul(ps[:], weff[:, i * COUT:(i + 1) * COUT],
                                  xbuf[:, o:o + NOUT], start=False, stop=(i == 2))
        mm.then_inc(s_mm, 1)

        # ---- post: one full-width clip straight out of PSUM (a
        # single op pays the DVE PSUM-access overhead once); both
        # out-DMA halves issue together on SP/ACT (fini drains cover
        # completion) ----
        nc.vector.wait_ge(s_mm, 1)
        nc.vector.tensor_scalar(v0[:], ps[:], AMAX, AMIN, AL.min, AL.max).then_inc(s_clip, 1)
        nc.sync.wait_ge(s_clip, 1)
        nc.sync.dma_start(yout[:, 0:NH], v0[:, 0:NH]).then_inc(s_wt, 16)
        nc.scalar.wait_ge(s_clip, 1)
        nc.scalar.dma_start(yout[:, NH:NOUT], v0[:, NH:NOUT]).then_inc(s_wb, 16)

    # Strip the framework const-AP memsets and the post-init all-engine
    # barrier (they are unused here; HW semaphores are zero at NEFF load
    # and re-zeroed by the NEFF epilogue). Only the construction-time
    # preamble prefix is touched.
    insts = main.instructions
    pre = [
        ins for ins in insts[:n_preamble]
        if type(ins).__name__ not in (
            "InstMemset", "InstDrain", "InstEventSemaphore", "InstRegisterMove")
    ]
    main.instructions = pre + insts[n_preamble:]

    return nc


def _get_nc():
    global _CACHED
    if _CACHED is None:
        _CACHED = _build()
    return _CACHED


def _shard_inputs(x, weight):
    xpad = np.pad(np.ascontiguousarray(x, dtype=np.float32),
                  ((0, 0), (0, 0), (1, 1), (1, 1)))
    wre = np.asarray(weight, dtype=np.float32).transpose(1, 2, 3, 0)  # [ci, kh, kw, co]
    wtap = [wre[:, t // 3, t % 3, :] for t in range(9)]               # each [C, COUT]
    w_top = np.concatenate([wtap[0], wtap[1], wtap[3], wtap[2], wtap[6], wtap[8]], axis=1)
    w_bot = np.concatenate([wtap[4], wtap[5], wtap[7]], axis=1)
    in_maps = []
    for c in range(8):
        b, q = divmod(c, 4)
        top = xpad[b, :, RPC * q:RPC * q + SECR, :].reshape(C, LEN)
        xw = np.zeros((2 * C, NIN), np.float32)
        xw[0:C, 0:LEN] = top
        xw[C:2 * C, 0:LEN - 35] = top[:, 35:]
        xw[0:C, LEN:NIN] = w_top
        xw[C:2 * C, LEN:LEN + NWB] = w_bot
        in_maps.append({"xw": np.ascontiguousarray(xw)})
    return in_maps


def kernel(x, weight):
    nc = _get_nc()
    in_maps = _shard_inputs(x, weight)
    res = run_bass_kernel_spmd(nc, in_maps, core_ids=list(range(8)))
    out = np.empty((B, COUT, H, W), dtype=np.float32)
    for c in range(8):
        b, q = divmod(c, 4)
        y = np.asarray(res.results[c]["y"], dtype=np.float32)  # exact f16 upcast
        for r in range(RPC):
            out[b, :, RPC * q + r, :] = y[:, r * SECW:r * SECW + W]
    return out


# revision 49
# speedup vs baseline: 1.1421x; 1.0343x over previous
"""Trainium2 Bass kernel for nn_Conv2d_mvm (crossbar-quantized 3x3 conv).

The reference simulates a bit-sliced crossbar. Reductions:

1. The ADC clip [0, 511] can never bind (max per-xbar analog sum is
   128 rows * max slice digit 3 = 384), so the computation is exactly
   linear in the bit decompositions.

2. The weight reconstruction applies slice_w[0] = -2^14 to the whole
   MSB 2-bit digit, which is NOT true 2's complement: net effect the
   conv uses effective weights  w_eff = wi - 32768*[wi < 0]  with
   wi = rne(4096*w), and xi = rne(4096*x) exactly.

3. Precision slack (gate is rel_err < 2e-2; measured ~1.4e-3): skip
   integer rounding and fold the final /2^24 into the quant scales --
   fp16 is scale-invariant under powers of two, so
     xbuf = fp16(x)            (== fp16(4096 x) / 4096 exactly)
     weff = fp16(w - 8*[w < -1/8192])
   makes PSUM hold the final pre-clip value directly: no post scale
   pass; the DVE clips straight out of PSUM. fp16(x) is produced by
   the SWDGE cast-during-DMA, so the ACT engine runs no compute at
   all (no activation-table load on the critical path).

Implementation (8 cores, data-parallel over batch x row-blocks):
  - core c handles batch c//4, output rows 8*(c%4) .. 8*(c%4)+8.
  - K=128 tap packing: x lands twice -- partitions 0-63 the padded
    [10, 34] section flat, partitions 64-127 the same flat buffer
    shifted 35 columns (one row + one col). Taps with OFFS delta 35
    share one K=128 matmul (top half hits offset o, bottom o+35):
    pairs (0,4), (1,5), (3,7); taps 2, 6, 8 stay top-half K=64.
    6 matmul slots instead of 9, singles first (they only need the
    top x half + top-half weights, which land earliest).
    (A 5-slot variant with taps 6/7 as concurrent (0,0)/(64,0)
    row-tiles compiled but died at runtime -- base-partition-64
    K=64 matmuls are not usable.)
  - weights [128, 384] f32: col blocks 0-2 hold the pairs stacked
    top/bottom, blocks 3-5 taps 2, 6, 8 in the top half only.
  - DMA split: ACT carries w_top [64,384] (its doorbell is ~0.8us
    ahead of SP's, which sits behind a walrus-inserted drain), SP
    w_bot [64,192], GpSimd the two x cast-DMAs (f32 DRAM -> f16
    SBUF, top then bottom; SWDGE is the only caster). Separate
    semaphores per piece so consumers wait only for what they read.
  - DVE: mneg = -8*[w < -1/8192] (fp32 w -- the threshold must be
    evaluated pre-fp16 or ~5 mask bits flip), weff = fp16(w + mneg),
    singles chunk first (top w only), pairs chunk second. The x-cast
    waits live on the DVE behind each weff chunk (a tiny 8-col op
    carries the s_dve release after the x wait; then_inc directly on
    a wait_ge fires the set without honoring the wait). The PE only
    watches s_dve. After the matmuls the DVE clips
    v0 = fp16(min(max(psum, -8), 32767/4096)) straight out of PSUM
    in one full-width op (a single op pays the DVE PSUM-access
    overhead once); both out-DMA halves then issue together on
    SP/ACT (fini drains cover completion; the host upcast of the
    f16 output is exact, and saturated values -- ~95% of them --
    are f16-exact). GpSimd cannot read PSUM, so the clip cannot be
    split across engines.
  - PE warm-up: 7 garbage-input K=128 N=512 matmuls issue as the
    PE's first instructions (~3.1us at the cold 1.2GHz clock), so the
    HAM activity window un-throttles the PE clock to 2.4GHz right as
    the real matmuls (which start ~3.3us after main-block entry)
    begin. They read never-written SBUF into a scratch PSUM bank --
    no memsets or semaphores needed. (8 dummies overshoot and delay
    the real matmuls; fewer leave an idle hole that loses the ramp.)

All value arithmetic happens on device; the host only pads, shards,
reshapes and gathers.
"""

from contextlib import ExitStack

import numpy as np

import concourse.bass as bass
import concourse.mybir as mybir
from concourse.bass_utils import run_bass_kernel_spmd

# fixed problem shape
B, C, H, W = 2, 64, 32, 32
COUT = 64
RPC = 8                    # output rows per core
SECR = RPC + 2             # padded rows per section
SECW = W + 2               # padded width
LEN = SECR * SECW          # 340
NOUT = (RPC - 1) * SECW + W  # 270 psum columns covering all valid pixels
NW = 6 * COUT              # 384 packed weight cols (3 pairs + 3 singles)
NWB = 3 * COUT             # 192 bottom-half weight cols (pairs only)
NIN = LEN + NW             # 724 packed input columns per partition
NH = 136                   # out-DMA column split

AMAX = 32767.0 / 4096.0
AMIN = -8.0
THR = -1.0 / 8192.0        # wi = rne(4096 w) < 0  <=>  w < -1/8192

F32 = mybir.dt.float32
F16 = mybir.dt.float16

# The NEFF fini block resets every HW semaphore below the compiler's
# max-sem-num bound. Packing bass's kernel semaphores just above
# walrus's internal ones and telling walrus the bound keeps the
# program's semaphore footprint minimal.
MAX_SEM = 64


def _patch_sem_budget():
    import concourse.bass_utils as bu
    if getattr(bu, "_sem_budget_patched", False):
        return
    bass.get_walrus_max_sem_num = lambda: MAX_SEM - 8
    orig = bu.get_walrus_args

    def patched(*a, **k):
        # ldw-opt dedupes the identical LDWEIGHTS of the 7 warm-up
        # matmuls (smaller PE iram stream; real matmuls all have
        # distinct weights and are unaffected)
        return [*orig(*a, **k), f"--max-sem-num={MAX_SEM}",
                "--enable-ldw-opt=true"]

    bu.get_walrus_args = patched
    bu._sem_budget_patched = True


_CACHED = None


def _build():
    _patch_sem_budget()
    nc = bass.Bass("TRN2", target_bir_lowering=False, debug=False, num_devices=8,
                   monotonic_sem_count=0)
    main = nc.m.functions[0].blocks[0]
    assert main.name == "main"
    n_preamble = len(main.instructions)

    xwin = nc.dram_tensor("xw", [2 * C, NIN], F32, kind="ExternalInput").ap()
    yout = nc.dram_tensor("y", [COUT, NOUT], F16, kind="ExternalOutput").ap()

    with ExitStack() as ctx:
        wraw = ctx.enter_context(nc.sbuf_tensor([2 * C, NW], F32))
        xbuf = ctx.enter_context(nc.sbuf_tensor([2 * C, LEN], F16))
        mneg = ctx.enter_context(nc.sbuf_tensor([2 * C, NW], F16))
        weff = ctx.enter_context(nc.sbuf_tensor([2 * C, NW], F16))
        v0 = ctx.enter_context(nc.sbuf_tensor([COUT, NOUT], F16))
        wdum = ctx.enter_context(nc.sbuf_tensor([2 * C, COUT], F16))
        mdum = ctx.enter_context(nc.sbuf_tensor([2 * C, 512], F16))
        ps = ctx.enter_context(nc.psum_tensor([COUT, NOUT], F32))
        psd = ctx.enter_context(nc.psum_tensor([COUT, 512], F32))
        s_wt = ctx.enter_context(nc.semaphore())
        s_wb = ctx.enter_context(nc.semaphore())
        s_xt = ctx.enter_context(nc.semaphore())
        s_xb = ctx.enter_context(nc.semaphore())
        s_dve = ctx.enter_context(nc.semaphore())
        s_mm = ctx.enter_context(nc.semaphore())
        s_clip = ctx.enter_context(nc.semaphore())

        AL = mybir.AluOpType

        # ---- input DMAs: w halves on the two HWDGE queues (w_top on
        # the ACT queue -- the SP engine's first doorbell is ~0.7us
        # late behind a walrus-inserted drain), x via SWDGE cast-DMA
        # (f32 -> f16), top half first ----
        nc.scalar.dma_start(wraw[0:C, :], xwin[0:C, LEN:NIN]).then_inc(s_wt, 16)
        nc.sync.dma_start(wraw[C:2 * C, 0:NWB], xwin[C:2 * C, LEN:LEN + NWB]).then_inc(s_wb, 16)
        nc.gpsimd.dma_start(xbuf[0:C, :], xwin[0:C, 0:LEN]).then_inc(s_xt, 16)
        # bottom copy: pairs read cols 34..303 only
        nc.gpsimd.dma_start(xbuf[C:2 * C, 0:304], xwin[C:2 * C, 0:304]).then_inc(s_xb, 16)

        # ---- DVE weight chain: singles chunk (top-only) first. The
        # x-cast waits also live here so the PE only watches s_dve ----
        nc.vector.wait_ge(s_wt, 16)
        nc.vector.tensor_scalar(mneg[0:C, NWB:NW], wraw[0:C, NWB:NW], THR, -8.0, AL.is_lt, AL.mult)
        nc.vector.scalar_tensor_tensor(weff[0:C, NWB:NW], wraw[0:C, NWB:NW], 1.0, mneg[0:C, NWB:NW], AL.mult, AL.add)
        nc.vector.wait_ge(s_xt, 16)
        nc.vector.tensor_scalar(mneg[0:1, 0:8], wraw[0:1, 0:8], THR, -8.0, AL.is_lt, AL.mult).then_inc(s_dve, 1)
        nc.vector.wait_ge(s_wb, 16)
        nc.vector.tensor_scalar(mneg[:, 0:NWB], wraw[:, 0:NWB], THR, -8.0, AL.is_lt, AL.mult)
        nc.vector.scalar_tensor_tensor(weff[:, 0:NWB], wraw[:, 0:NWB], 1.0, mneg[:, 0:NWB], AL.mult, AL.add)
        nc.vector.wait_ge(s_xb, 16)
        nc.vector.tensor_scalar(mneg[0:1, 0:8], wraw[0:1, 0:8], THR, -8.0, AL.is_lt, AL.mult).then_inc(s_dve, 1)

        # ---- PE: warm-up dummies, then 6 real matmul slots ----
        NDUM = 7
        for i in range(NDUM):
            nc.tensor.matmul(psd[:], wdum[:, 0:COUT], mdum[:],
                             start=(i == 0), stop=(i == NDUM - 1))

        nc.tensor.wait_ge(s_dve, 1)
        for i, o in enumerate((2, 68, 70)):      # taps 2, 6, 8
            nc.tensor.matmul(ps[:], weff[0:C, (3 + i) * COUT:(4 + i) * COUT],
                             xbuf[0:C, o:o + NOUT], start=(i == 0), stop=False)
        nc.tensor.wait_ge(s_dve, 2)
        for i, o in enumerate((0, 1, 34)):       # pairs (0,4), (1,5), (3,7)
            mm = nc.tensor.matmul(ps[:], weff[:, i * COUT:(i + 1) * COUT],
                                  xbuf[:, o:o + NOUT], start=False, stop=(i == 2))
        mm.then_inc(s_mm, 1)

        # ---- post: one full-width clip straight out of PSUM (a
        # single op pays the DVE PSUM-access overhead once); both
        # out-DMA halves issue together on SP/ACT (fini drains cover
        # completion) ----
        nc.vector.wait_ge(s_mm, 1)
        nc.vector.tensor_scalar(v0[:], ps[:], AMAX, AMIN, AL.min, AL.max).then_inc(s_clip, 1)
        # one full-width out-DMA: issue cost is per-descriptor
        # (= partitions), so the former two-way split only doubled
        # the queue-drain exposure on the barrier gate
        nc.sync.wait_ge(s_clip, 1)
        nc.sync.dma_start(yout[:], v0[:]).then_inc(s_wt, 16)

    # Strip the framework const-AP memsets and the post-init all-engine
    # barrier (they are unused here; HW semaphores are zero at NEFF load
    # and re-zeroed by the NEFF epilogue). Only the construction-time
    # preamble prefix is touched.
    insts = main.instructions
    pre = [
        ins for ins in insts[:n_preamble]
        if type(ins).__name__ not in (
            "InstMemset", "InstDrain", "InstEventSemaphore", "InstRegisterMove")
    ]
    main.instructions = pre + insts[n_preamble:]

    return nc


def _get_nc():
    global _CACHED
    if _CACHED is None:
        _CACHED = _build()
    return _CACHED


def _shard_inputs(x, weight):
    xpad = np.pad(np.ascontiguousarray(x, dtype=np.float32),
                  ((0, 0), (0, 0), (1, 1), (1, 1)))
    wre = np.asarray(weight, dtype=np.float32).transpose(1, 2, 3, 0)  # [ci, kh, kw, co]
    wtap = [wre[:, t // 3, t % 3, :] for t in range(9)]               # each [C, COUT]
    w_top = np.concatenate([wtap[0], wtap[1], wtap[3], wtap[2], wtap[6], wtap[8]], axis=1)
    w_bot = np.concatenate([wtap[4], wtap[5], wtap[7]], axis=1)
    in_maps = []
    for c in range(8):
        b, q = divmod(c, 4)
        top = xpad[b, :, RPC * q:RPC * q + SECR, :].reshape(C, LEN)
        xw = np.zeros((2 * C, NIN), np.float32)
        xw[0:C, 0:LEN] = top
        xw[C:2 * C, 0:LEN - 35] = top[:, 35:]
        xw[0:C, LEN:NIN] = w_top
        xw[C:2 * C, LEN:LEN + NWB] = w_bot
        in_maps.append({"xw": np.ascontiguousarray(xw)})
    return in_maps


def kernel(x, weight):
    nc = _get_nc()
    in_maps = _shard_inputs(x, weight)
    res = run_bass_kernel_spmd(nc, in_maps, core_ids=list(range(8)))
    out = np.empty((B, COUT, H, W), dtype=np.float32)
    for c in range(8):
        b, q = divmod(c, 4)
        y = np.asarray(res.results[c]["y"], dtype=np.float32)  # exact f16 upcast
        for r in range(RPC):
            out[b, :, RPC * q + r, :] = y[:, r * SECW:r * SECW + W]
    return out
